# revision 10
# baseline (speedup 1.0000x reference)
"""Trainium2 Bass kernel for nn_Conv2d_60009283059961.

Single-channel 2D cross-correlation, 8192x8192 image, 7x7 kernel, stride 2,
padding 3, plus scalar bias -> 4096x4096 output.

Strategy
--------
Row-shard the output across 8 NeuronCores (512 output rows each). Each core
receives a pre-padded input slab (its 1029 needed input rows + zero padding,
so no edge special-casing on device; the "halo exchange" is done host-side by
overlapping the slabs).

On each core the conv is computed on the TensorEngine as a banded matmul:
for a block of 61 output rows, K=128 consecutive input rows sit on SBUF
partitions, and for each of the 7 kernel-column shifts j we matmul
  psum[m, n] += sum_k  band_j[k, m] * x[k, 2n + j]
where band_j[k, m] = w[k - 2m, j] (zero outside 0 <= k-2m < 7).  The rhs is a
stride-2 free-dim view of the input tile; accumulating the 7 shifts in PSUM
yields the full 7x7 conv.  Operands use the float32r matmul mode (fp32 data,
~11-bit mantissa multiply, fp32 PSUM accumulate) which streams at 1 col/cycle
instead of fp32's 1/4 rate.  PSUM is drained through the VectorEngine with a
fused scalar bias add.
"""

import numpy as np

import concourse.bass as bass
import concourse.tile as tile
from concourse import mybir
from concourse.bass_utils import run_bass_kernel_spmd

# Problem constants (hardcoded per contract; kernel.py must be self-contained).
H = 8192          # input rows
W = 8192          # input cols
KH = KW = 7
STRIDE = 2
PAD = 3
OH = H // STRIDE  # 4096
OW = W // STRIDE  # 4096
NCORES = 8
RPC = OH // NCORES        # 512 output rows per core

MBLK = 61                 # output rows per PE block (2*61+5 <= 128)
NBLK = 512                # output cols per matmul (PSUM bank = 512 f32)
NROWBLK = (RPC + MBLK - 1) // MBLK    # 9 row blocks per core
NCOLBLK = OW // NBLK                  # 8 col tiles per core

SLAB_H = 1032             # per-core input slab rows (1029 used + pad)
SLAB_W = 8200             # per-core input slab cols (8197 used + pad)

LAST_RESULTS = None       # test.py introspection hook
LAST_NC = None            # built Bass program, for cost-model timing


def _split_excess_waits(nc, max_waits=1):
    """Workaround: this walrus build allows only one sync wait per
    instruction; spread extra waits across NOPs on the same engine."""
    for fn in nc.m.functions:
        for bb in fn.blocks:
            new = []
            for inst in bb.instructions:
                si = getattr(inst, "sync_info", None)
                if si is not None and si.on_wait is not None and len(si.on_wait) > max_waits:
                    waits = list(si.on_wait)
                    excess, keep = waits[:-max_waits], waits[-max_waits:]
                    for j in range(0, len(excess), max_waits):
                        new.append(mybir.InstNoOp(
                            name=nc.get_next_instruction_name(),
                            sync_info=mybir.SyncInfo(
                                on_wait=excess[j:j + max_waits], on_update=[]),
                            bass_nofuse=True,
                            engine=inst.engine,
                        ))
                    si.on_wait = keep
                new.append(inst)
            bb.instructions[:] = new


def _build_program(bias_val: float, xbufs=18, obufs=8, pbufs=8):
    f32 = mybir.dt.float32
    f32r = mybir.dt.float32r

    nc = bass.Bass("TRN2", target_bir_lowering=False, debug=False,
                   num_devices=NCORES)
    x_dram = nc.dram_tensor("xs", [SLAB_H, SLAB_W], f32r, kind="ExternalInput").ap()
    w_dram = nc.dram_tensor("wb", [128, 7 * 64], f32r, kind="ExternalInput").ap()
    out_dram = nc.dram_tensor("out", [RPC, OW], f32, kind="ExternalOutput").ap()

    CHW = 2 * NBLK + 8        # input chunk width: 1024 cols + 5 halo, padded

    def block_dims(b):
        m0 = b * MBLK
        return m0, min(MBLK, RPC - m0), min(128, SLAB_H - 2 * m0)

    from contextlib import ExitStack
    with tile.TileContext(nc) as tc, ExitStack() as ctx:
        wpool = ctx.enter_context(tc.tile_pool(name="w", bufs=1))
        xpool = ctx.enter_context(tc.tile_pool(name="x", bufs=xbufs))
        opool = ctx.enter_context(tc.tile_pool(name="o", bufs=obufs))
        ppool = ctx.enter_context(tc.tile_pool(name="p", bufs=pbufs, space="PSUM"))

        w_sb = wpool.tile([128, 7 * 64], f32r)
        nc.sync.dma_start(w_sb[:], w_dram[:])

        chunks = {}

        def load_chunk(b, t):
            # One independent [128, CHW] tile per (block, col-tile); group
            # (b, t) depends only on its own chunk, and chunk DMAs emitted
            # ahead of compute get program-order priority over output DMAs.
            if b >= NROWBLK:
                return
            m0, mb, kb = block_dims(b)
            ch = xpool.tile([128, CHW], f32r, tag="xchunk")
            c0 = 1024 * t
            cw = min(CHW, SLAB_W - c0)
            # SWDGE for inputs keeps descriptor generation off the HWDGE path
            # that the (latency-sensitive) output stores use.
            nc.gpsimd.dma_start(ch[0:kb, 0:cw], x_dram[2 * m0:2 * m0 + kb, c0:c0 + cw])
            chunks[(b, t)] = ch

        WINDOW = 2  # blocks of chunk prefetch beyond the current one
        for b in range(WINDOW):
            for t in range(NCOLBLK):
                load_chunk(b, t)

        for b in range(NROWBLK):
            m0, mb, kb = block_dims(b)
            for t in range(NCOLBLK):
                load_chunk(b + WINDOW, t)
                ch = chunks.pop((b, t))
                p = ppool.tile([64, NBLK], f32)
                for j in range(KW):
                    rhs = ch[0:kb, j: j + 2 * NBLK: 2]
                    lhsT = w_sb[0:kb, 64 * j: 64 * j + mb]
                    nc.tensor.matmul(p[0:mb, :], lhsT, rhs,
                                     start=(j == 0), stop=(j == KW - 1))
                outsb = opool.tile([MBLK, NBLK], f32)
                nc.vector.tensor_scalar_add(outsb[0:mb, :], p[0:mb, :], bias_val)
                nc.sync.dma_start(
                    out_dram[m0:m0 + mb, t * NBLK:(t + 1) * NBLK], outsb[0:mb, :])

    _split_excess_waits(nc)
    return nc


def kernel(enc_x, weight, bias, num_row, num_col):
    global LAST_RESULTS
    enc_x = np.asarray(enc_x, dtype=np.float32)
    weight = np.asarray(weight, dtype=np.float32).reshape(KH, KW)
    bias_val = float(np.asarray(bias).reshape(-1)[0])
    assert int(num_row) == H and int(num_col) == W

    x = enc_x.reshape(H, W)

    # Per-core input slabs with halo + zero padding baked in.
    # Core c computes output rows [512c, 512c+512); output row r reads input
    # rows [2r-3, 2r+3].  Slab local row li <-> global row g = 1024c - 3 + li.
    in_maps = []
    wband = np.zeros((128, 7 * 64), dtype=np.float32)
    for k in range(128):
        for m in range(min(MBLK, (k // 2) + 4)):
            i = k - 2 * m
            if 0 <= i < KH:
                for j in range(KW):
                    wband[k, 64 * j + m] = weight[i, j]

    for c in range(NCORES):
        slab = np.zeros((SLAB_H, SLAB_W), dtype=np.float32)
        g0 = 1024 * c - 3
        src_lo = max(0, g0)
        src_hi = min(H, g0 + 1029)
        slab[src_lo - g0:src_hi - g0, 3:3 + W] = x[src_lo:src_hi, :]
        in_maps.append({"xs": slab, "wb": wband})

    global LAST_NC
    nc = _build_program(bias_val)
    LAST_NC = nc
    res = run_bass_kernel_spmd(nc, in_maps, core_ids=list(range(NCORES)))
    LAST_RESULTS = res

    out = np.concatenate([res.results[c]["out"] for c in range(NCORES)], axis=0)
    return out.reshape(-1)
